# revision 1
# baseline (speedup 1.0000x reference)
"""Block-circulant process via frequency-domain factorization on 8 cores.

out = x @ M factorizes through the (truncated, 48-bin) real FFT:
  stage A: per in-block j:  S[(p,f), b] = sum_t F[t,(p,f)] xT[jB+t, b]
  stage M: per freq pair e: mid[(q,i), b] = sum_{p,j} W_e[(p,j),(q,i)] S
  stage C: per out-block i: out[t, b] = sum_{q,f} G[(q,f), t] mid

All stages are single K<=128 matmuls (no PSUM accumulation). The two
partition-regroups between stages bounce through DRAM with affine
scatter APs. Sharding: pure data-parallel over batch (x dim 0), all
weight operands replicated. fp32r throughout.

PE per core: 88 matmuls (~20us). HBM per core: ~41 MiB.
"""

import numpy as np

B = 128
K_HALF = B // 2 + 1  # 65
KT = 48  # frequency truncation
KI = 32
KO = 32
BATCH = 4096
IN_F = 4096
OUT_F = 4096

N_CORES = 8
BQ = BATCH // N_CORES  # 512 batch rows per core
NP = KT // 2  # 24 frequency pairs
FE = NP  # e index range

_CACHE = {}
LAST_RESULTS = None
TRACE = False


def _build_nc():
    import concourse.bacc as bacc
    import concourse.mybir as mybir
    import concourse.tile as tile

    F32R = mybir.dt.float32r
    F32 = mybir.dt.float32

    nc = bacc.Bacc(None, target_bir_lowering=False)
    xT = nc.declare_dram_parameter("xT", [IN_F, BQ], F32R, isOutput=False)
    fmat = nc.declare_dram_parameter("fmat", [128, 96], F32R, isOutput=False)
    gmat = nc.declare_dram_parameter("gmat", [96, 128], F32R, isOutput=False)
    wmid = nc.declare_dram_parameter("wmid", [128, NP * 128], F32R,
                                     isOutput=False)
    oT = nc.declare_dram_parameter("oT", [OUT_F, BQ], F32, isOutput=True)

    # DRAM intermediates, laid out so stages M and C each load their whole
    # input with ONE contiguous DMA (48/64KB partition lines)
    # sS[fl*64 + p*32 + j, e*BQ + b]
    sS = nc.dram_tensor("sS", [128, NP * BQ], F32R)
    # cmid[q*48 + f, i*BQ + b]
    cmid = nc.dram_tensor("cmid", [96, KO * BQ], F32R)

    # views for the scattered writes
    sS_v = sS.rearrange("(fl p j) (e b) -> fl j p e b", fl=2, p=2, e=NP)
    cmid_v = cmid.rearrange("(q fe fl) (i b) -> fl fe q i b", fl=2, fe=FE,
                            i=KO)

    with tile.TileContext(nc) as tc:
        with (
            tc.tile_pool(name="cpool", bufs=1) as cpool,
            tc.tile_pool(name="xpool", bufs=8) as xpool,
            tc.tile_pool(name="spool", bufs=24) as spool,
            tc.tile_pool(name="bigpool", bufs=3) as bigpool,
            tc.tile_pool(name="opool", bufs=10) as opool,
            tc.tile_pool(name="psum", bufs=3, space="PSUM") as psum,
            tc.tile_pool(name="psum2", bufs=2, space="PSUM") as psum2,
        ):
            f_t = cpool.tile([128, 96], F32R, name="f_t")
            nc.sync.dma_start(f_t[:], fmat[:])
            g_t = cpool.tile([96, 128], F32R, name="g_t")
            nc.sync.dma_start(g_t[:], gmat[:])
            # all 24 middle weight blocks in one DMA
            w_all = cpool.tile([128, NP * 128], F32R, name="w_all")
            nc.sync.dma_start(w_all[:], wmid[:])

            lanes = [nc.scalar, nc.sync, nc.gpsimd]

            # ---- stage A: 32 matmuls + scattered DRAM writes ----
            for j in range(KI):
                x_t = xpool.tile([128, BQ], F32R, name="x_t")
                (nc.sync if j % 2 == 0 else nc.scalar).dma_start(
                    x_t[:], xT[j * 128:(j + 1) * 128, :])
                ps = psum.tile([96, BQ], mybir.dt.float32, name="ps_a",
                               tag="ps_a")
                nc.tensor.matmul(ps[:], f_t[:], x_t[:], start=True, stop=True)
                s_t = spool.tile([96, BQ], F32, name="s_t")
                nc.vector.tensor_copy(s_t[:], ps[:])
                for fl in range(2):
                    nc.gpsimd.dma_start(
                        sS_v[fl, j],
                        s_t[fl * 48:(fl + 1) * 48, :].bitcast(F32R),
                    )

            # ---- stage M: grouped reads (6 pairs/DMA) + 24 matmuls ----
            EG = 6
            for g in range(NP // EG):
                m_g = bigpool.tile([128, EG * BQ], F32R, name="m_g",
                                   tag="big")
                nc.sync.dma_start(m_g[:], sS[:, g * EG * BQ:(g + 1) * EG * BQ])
                for ee in range(EG):
                    e = g * EG + ee
                    ps = psum2.tile([128, BQ], mybir.dt.float32,
                                    name="ps_m", tag="ps_m")
                    nc.tensor.matmul(ps[:], w_all[:, e * 128:(e + 1) * 128],
                                     m_g[:, ee * BQ:(ee + 1) * BQ],
                                     start=True, stop=True)
                    m_out = opool.tile([128, BQ], F32, name="m_out", tag="mo")
                    nc.vector.tensor_copy(m_out[:], ps[:])
                    for fl in range(2):
                        (nc.scalar if fl == 0 else nc.gpsimd).dma_start(
                            cmid_v[fl, e],
                            m_out[fl * 64:(fl + 1) * 64, :].bitcast(F32R),
                        )

            # ---- stage C: grouped reads (8 i/DMA) + 32 matmuls ----
            IG = 8
            for g in range(KO // IG):
                c_g = bigpool.tile([96, IG * BQ], F32R, name="c_g", tag="big")
                nc.sync.dma_start(c_g[:],
                                  cmid[:, g * IG * BQ:(g + 1) * IG * BQ])
                for ii in range(IG):
                    i = g * IG + ii
                    ps = psum.tile([128, BQ], mybir.dt.float32, name="ps_c",
                                   tag="ps_c")
                    nc.tensor.matmul(ps[:], g_t[:],
                                     c_g[:, ii * BQ:(ii + 1) * BQ],
                                     start=True, stop=True)
                    o_t = opool.tile([128, BQ], F32, name="o_t")
                    nc.vector.tensor_copy(o_t[:], ps[:])
                    (nc.scalar if i % 2 == 0 else nc.gpsimd).dma_start(
                        oT[i * 128:(i + 1) * 128, :], o_t[:])
    nc.finalize()
    return nc


def _get_nc():
    if "nc" not in _CACHE:
        _CACHE["nc"] = _build_nc()
    return _CACHE["nc"]


def _host_weights(W_real, W_imag):
    """F [128,96], G [96,128], Wmid [24,128,128] (all float32)."""
    t = np.arange(B)[:, None].astype(np.float64)
    # F columns ordered (fl, p, e): f = 2e + fl; p=0 -> cos, p=1 -> -sin
    F = np.zeros((128, 96))
    for fl in range(2):
        for p in range(2):
            for e in range(FE):
                f = 2 * e + fl
                col = fl * 48 + p * 24 + e
                w = 2 * np.pi * f * t[:, 0] / B
                F[:, col] = np.cos(w) if p == 0 else -np.sin(w)
    # G rows ordered (q, f): q=0 -> scale*cos, q=1 -> -scale*sin
    G = np.zeros((96, 128))
    fs = np.arange(KT)
    scale = np.full(KT, 2.0 / B)
    scale[0] = 1.0 / B
    for q in range(2):
        for f in range(KT):
            w = 2 * np.pi * f * np.arange(B) / B
            G[q * 48 + f] = (scale[f] * np.cos(w) if q == 0
                             else -scale[f] * np.sin(w))
    # Wmid[e]: rows (fl, p, j), cols (fl, q, i); block-diag in fl
    Wr = W_real.astype(np.float64)
    Wi = W_imag.astype(np.float64)
    Wm = np.zeros((NP, 128, 128))
    for e in range(NP):
        for fl in range(2):
            f = 2 * e + fl
            r0, c0 = fl * 64, fl * 64
            # q=0: Re_out = Wr @ Re + Wi @ Im ; q=1: Im_out = Wr @ Im - Wi @ Re
            # rows (p=0: Re-in j), (p=1: Im-in j); cols (q, i)
            # lhsT[(p,j),(q,i)]: value multiplying S[p,j] into out[q,i]
            Wrf = Wr[:, :, f].T  # [j, i]
            Wif = Wi[:, :, f].T
            Wm[e, r0:r0 + 32, c0:c0 + 32] = Wrf          # p0 -> q0: Wr
            Wm[e, r0 + 32:r0 + 64, c0:c0 + 32] = Wif     # p1 -> q0: Wi
            Wm[e, r0:r0 + 32, c0 + 32:c0 + 64] = -Wif    # p0 -> q1: -Wi
            Wm[e, r0 + 32:r0 + 64, c0 + 32:c0 + 64] = Wrf  # p1 -> q1: Wr
    return (F.astype(np.float32), G.astype(np.float32), Wm.astype(np.float32))


def kernel(x, W_real, W_imag):
    global LAST_RESULTS
    from concourse.bass_utils import run_bass_kernel_spmd

    x = np.asarray(x, dtype=np.float32)
    F, G, Wm = _host_weights(np.asarray(W_real), np.asarray(W_imag))
    xt = np.ascontiguousarray(x.T)  # (IN_F, BATCH)

    in_maps = []
    for core in range(N_CORES):
        xT_shard = np.ascontiguousarray(xt[:, core * BQ:(core + 1) * BQ])
        wm_packed = np.ascontiguousarray(
            Wm.transpose(1, 0, 2).reshape(128, NP * 128))
        in_maps.append(
            {"xT": xT_shard, "fmat": F, "gmat": G, "wmid": wm_packed})

    nc = _get_nc()
    res = run_bass_kernel_spmd(nc, in_maps, list(range(N_CORES)), trace=TRACE)
    LAST_RESULTS = res

    out = np.empty((BATCH, OUT_F), np.float32)
    for core in range(N_CORES):
        out[core * BQ:(core + 1) * BQ, :] = res.results[core]["oT"].T
    return out



# revision 2
# speedup vs baseline: 1.0190x; 1.0190x over previous
"""Block-circulant process via truncated real-FFT factorization, v3.

out = x @ M through the 48-bin real FFT (B=128 blocks), all fp16,
fully SBUF-resident:
  stage A: sA[(c,e), (j,b)]  = F^T x_j       32 MMs, K=128   c=(fl,p)
  turn 1:  S2[(c,j), (e,b)]  = corner-turn   24 per-e SBUF DMAs
  stage M: mid[(c2,i),(e,b)] = Wm_e^T S2_e   24 MMs, K=128   c2=(q,fl)
  turn 2:  C2[(c2,e),(i,b)]  = corner-turn   32 per-i SBUF DMAs
  stage C: oT[t, (i,b)]      = G^T C2_i      32 MMs, K=96

Corner-turn calls write 96-128 partitions each (reads gather from 4
partitions on distinct AXI ports) so descriptors spread across all 16
SDMA engines and the HWDGE generator never backpressures. Host does
all layout permutes; x-in/out DMAs are partition-linear 8KB/partition.
Data-parallel over batch: 512 rows/core.
"""

import numpy as np

B = 128
KT = 48  # frequency truncation
KI = 32
KO = 32
BATCH = 4096
IN_F = 4096
OUT_F = 4096

N_CORES = 8
BQ = BATCH // N_CORES  # 512
NP = KT // 2  # 24 frequency pairs

_CACHE = {}
LAST_RESULTS = None
TRACE = False


def _build_nc():
    import concourse.bacc as bacc
    import concourse.mybir as mybir
    import concourse.tile as tile

    F16 = mybir.dt.float16
    F32 = mybir.dt.float32

    nc = bacc.Bacc(None, target_bir_lowering=False)
    xp = nc.declare_dram_parameter("xp", [128, KI * BQ], F16, isOutput=False)
    fg = nc.declare_dram_parameter("fg", [128, 256], F16, isOutput=False)
    wm = nc.declare_dram_parameter("wm", [128, NP * 128], F16, isOutput=False)
    op = nc.declare_dram_parameter("op", [128, KO * BQ], F16, isOutput=True)

    with tile.TileContext(nc) as tc:
        with (
            tc.tile_pool(name="cpool", bufs=1) as cpool,
            tc.tile_pool(name="psum", bufs=4, space="PSUM") as psum,
        ):
            fg_t = cpool.tile([128, 256], F16, name="fg_t")
            nc.gpsimd.dma_start(fg_t[:], fg[:])
            wm_t = cpool.tile([128, NP * 128], F16, name="wm_t")
            nc.gpsimd.dma_start(wm_t[:], wm[:])

            x_t = cpool.tile([128, KI * BQ], F16, name="x_t")
            chunks = [2, 2] + [4] * 7
            pos = 0
            for g, w in enumerate(chunks):
                (nc.sync if g % 2 == 0 else nc.scalar).dma_start(
                    x_t[:, pos * BQ:(pos + w) * BQ],
                    xp[:, pos * BQ:(pos + w) * BQ])
                pos += w

            sA = cpool.tile([96, KI * BQ], F16, name="sA")
            S2 = cpool.tile([128, NP * BQ], F16, name="S2")
            mid = cpool.tile([128, NP * BQ], F16, name="mid")
            C2 = cpool.tile([96, KO * BQ], F16, name="C2")
            oT = cpool.tile([128, KO * BQ], F16, name="oT")

            # corner-turn source views (4 partitions, distinct AXI ports)
            sAv = sA.rearrange("(c e) (j b) -> c e j b", c=4, j=KI)
            midv = mid.rearrange("(c i) (e b) -> c i e b", c=4, e=NP)

            f_ap = fg_t[:, 0:128]       # [t, (c,e)pad]  K=128
            g_ap = fg_t[0:96, 128:256]  # [(c2,e), t]    K=96

            # ---- PE warm-up during the x load (HAM clock ramp) ----
            for wn in range(24):
                ps = psum.tile([128, 2 * BQ], F32, name="ps_w", tag="ps")
                nc.tensor.matmul(ps[:, 0:256], f_ap, fg_t[:, 0:256],
                                 start=True, stop=True)

            # ---- stage A (paired MMs, one 2-bank copy per pair) ----
            for j in range(0, KI, 2):
                ps = psum.tile([128, 2 * BQ], F32, name="ps_a", tag="ps")
                nc.tensor.matmul(ps[:, 0:BQ], f_ap,
                                 x_t[:, j * BQ:(j + 1) * BQ],
                                 start=True, stop=True)
                nc.tensor.matmul(ps[:, BQ:2 * BQ], f_ap,
                                 x_t[:, (j + 1) * BQ:(j + 2) * BQ],
                                 start=True, stop=True)
                if (j // 2) % 2 == 0:
                    nc.vector.tensor_copy(sA[:, j * BQ:(j + 2) * BQ],
                                          ps[0:96, :])
                else:
                    nc.scalar.copy(sA[:, j * BQ:(j + 2) * BQ], ps[0:96, :])

            # ---- turn 1: per-e, dst spreads all 128 partitions ----
            for e in range(NP):
                (nc.sync if e % 2 == 0 else nc.gpsimd).dma_start(
                    S2[:, e * BQ:(e + 1) * BQ], sAv[:, e])

            for wn in range(4):
                ps = psum.tile([128, 2 * BQ], F32, name="ps_w2", tag="ps")
                nc.tensor.matmul(ps[:, 0:256], f_ap, fg_t[:, 0:256],
                                 start=True, stop=True)

            # ---- stage M ----
            for e in range(0, NP, 2):
                ps = psum.tile([128, 2 * BQ], F32, name="ps_m", tag="ps")
                nc.tensor.matmul(ps[:, 0:BQ], wm_t[:, e * 128:(e + 1) * 128],
                                 S2[:, e * BQ:(e + 1) * BQ],
                                 start=True, stop=True)
                nc.tensor.matmul(ps[:, BQ:2 * BQ],
                                 wm_t[:, (e + 1) * 128:(e + 2) * 128],
                                 S2[:, (e + 1) * BQ:(e + 2) * BQ],
                                 start=True, stop=True)
                if (e // 2) % 2 == 0:
                    nc.vector.tensor_copy(mid[:, e * BQ:(e + 2) * BQ], ps[:])
                else:
                    nc.scalar.copy(mid[:, e * BQ:(e + 2) * BQ], ps[:])

            # ---- turn 2: per-i, dst spreads 96 partitions; HWDGE pool
            # (sync+scalar share 4 drain engines) gets 14, gpsimd 18 ----
            for i in range(KO):
                if i % 16 in (1, 4, 7, 10, 13, 15, 2):
                    lane = nc.sync if i % 2 == 0 else nc.scalar
                else:
                    lane = nc.gpsimd
                lane.dma_start(C2[:, i * BQ:(i + 1) * BQ], midv[:, i])

            for wn in range(4):
                ps = psum.tile([128, 2 * BQ], F32, name="ps_w3", tag="ps")
                nc.tensor.matmul(ps[:, 0:256], f_ap, fg_t[:, 0:256],
                                 start=True, stop=True)

            # ---- stage C + output ----
            for i in range(0, KO, 2):
                ps = psum.tile([128, 2 * BQ], F32, name="ps_c", tag="ps")
                nc.tensor.matmul(ps[:, 0:BQ], g_ap,
                                 C2[:, i * BQ:(i + 1) * BQ],
                                 start=True, stop=True)
                nc.tensor.matmul(ps[:, BQ:2 * BQ], g_ap,
                                 C2[:, (i + 1) * BQ:(i + 2) * BQ],
                                 start=True, stop=True)
                if (i // 2) % 2 == 0:
                    nc.vector.tensor_copy(oT[:, i * BQ:(i + 2) * BQ], ps[:])
                else:
                    nc.scalar.copy(oT[:, i * BQ:(i + 2) * BQ], ps[:])
                if i % 8 == 6:
                    i0 = i - 6
                    (nc.sync if (i0 // 8) % 2 == 0 else nc.scalar).dma_start(
                        op[:, i0 * BQ:(i + 2) * BQ],
                        oT[:, i0 * BQ:(i + 2) * BQ])
    nc.finalize()
    return nc


def _get_nc():
    if "nc" not in _CACHE:
        _CACHE["nc"] = _build_nc()
    return _CACHE["nc"]


def _host_weights(W_real, W_imag):
    """F [128,128] (cols 96: zero), G [96,128], Wm [24,128,128] float64."""
    t = np.arange(B).astype(np.float64)
    # F columns (c, e) = (fl,p,e): col = fl*48 + p*24 + e; f = 2e+fl
    F = np.zeros((128, 128))
    for fl in range(2):
        for p in range(2):
            for e in range(NP):
                f = 2 * e + fl
                w = 2 * np.pi * f * t / B
                F[:, fl * 48 + p * 24 + e] = np.cos(w) if p == 0 else -np.sin(w)
    # G rows (c2, e) = (q,fl,e): row = (q*2+fl)*24 + e, f = 2e+fl;
    # q=0 -> scale*cos, q=1 -> -scale*sin
    G = np.zeros((96, 128))
    scale = np.full(KT, 2.0 / B)
    scale[0] = 1.0 / B
    for q in range(2):
        for fl in range(2):
            for e in range(NP):
                f = 2 * e + fl
                w = 2 * np.pi * f * t / B
                G[(q * 2 + fl) * 24 + e] = (scale[f] * np.cos(w) if q == 0
                                            else -scale[f] * np.sin(w))
    # Wm[e]: rows (fl, p, j) = fl*64+p*32+j; cols (q, fl, i) = q*64+fl*32+i
    Wr = W_real.astype(np.float64)
    Wi = W_imag.astype(np.float64)
    Wm = np.zeros((NP, 128, 128))
    for e in range(NP):
        for fl in range(2):
            f = 2 * e + fl
            r0 = fl * 64
            c0 = fl * 32
            Wrf = Wr[:, :, f].T  # [j, i]
            Wif = Wi[:, :, f].T
            Wm[e, r0:r0 + 32, c0:c0 + 32] = Wrf            # p0 -> q0: Wr
            Wm[e, r0 + 32:r0 + 64, c0:c0 + 32] = Wif       # p1 -> q0: Wi
            Wm[e, r0:r0 + 32, 64 + c0:64 + c0 + 32] = -Wif  # p0 -> q1: -Wi
            Wm[e, r0 + 32:r0 + 64, 64 + c0:64 + c0 + 32] = Wrf  # p1 -> q1
    return F, G, Wm


def kernel(x, W_real, W_imag):
    global LAST_RESULTS
    from concourse.bass_utils import run_bass_kernel_spmd

    x = np.asarray(x, dtype=np.float32)
    F, G, Wm = _host_weights(np.asarray(W_real), np.asarray(W_imag))
    fg_pack = np.zeros((128, 256), np.float16)
    fg_pack[:, 0:128] = F.astype(np.float16)
    fg_pack[0:96, 128:256] = G.astype(np.float16)
    wm_pack = np.ascontiguousarray(
        Wm.transpose(1, 0, 2)).reshape(128, NP * 128).astype(np.float16)
    x16 = x.astype(np.float16)

    in_maps = []
    for c in range(N_CORES):
        xs = x16[c * BQ:(c + 1) * BQ, :]  # [512, 4096]
        xpk = np.ascontiguousarray(
            xs.reshape(BQ, KI, B).transpose(2, 1, 0)).reshape(128, KI * BQ)
        in_maps.append({"xp": xpk, "fg": fg_pack, "wm": wm_pack})

    nc = _get_nc()
    res = run_bass_kernel_spmd(nc, in_maps, list(range(N_CORES)), trace=TRACE)
    LAST_RESULTS = res

    out = np.empty((BATCH, OUT_F), np.float32)
    for c in range(N_CORES):
        o = np.asarray(res.results[c]["op"])  # [128, KO*BQ] fp16
        out[c * BQ:(c + 1) * BQ, :] = (
            o.reshape(128, KO, BQ).transpose(2, 1, 0)
            .reshape(BQ, OUT_F).astype(np.float32))
    return out
